# revision 3
# baseline (speedup 1.0000x reference)
"""Trainium2 Bass kernel for nn_CT_37821482009425 (snntorch Leaky LIF scan).

Reference semantics (bitwise-matched):
    T = clip(t, 1, 5); x = roll(inp, roll_amount, axis=2)
    per step: reset = (mem > T); mem = 0.95*mem + x_t - reset*T; spk = (mem > T)
Output: spikes (1024, 1, 224, 224) float32 in {0, 1}.

Distribution: pure data parallelism - batch 1024 -> 8 cores x 128 (the SBUF
partition dim). Host prep per core: apply the roll and transpose to
time-major so each timestep's H=224 vector is contiguous per partition.

Per-core compute:
  * One custom DVE op per time step (LIF_STEP_ANT):
        out = (Src0*C0 + Src1) - (Src0 > C1)*C2
             = beta*m_prev + x_t - T*(m_prev > T)
    The reset feedback is re-derived from the membrane inside the op, so the
    whole serial step is a single 2-source DVE instruction (~294 ns at
    FD=224) instead of the 3-instruction stt/tt/ts chain. Rounding order
    matches the reference exactly (mul, add, exact mask*T, sub).
  * Spikes are extracted off the critical path on the scalar engine:
    Sign(m - T) written as uint8 per 16-step slab (spike <=> byte == 1),
    quartering output DMA traffic (input and output transfers serialize on
    the DMA engines, so output bytes are on the critical path).
  * sync engine streams x in 16-step slabs (4-deep ring), scalar engine
    writes spike slabs out (4-deep ring); membrane slabs ping-pong (x2).
"""

import numpy as np
import concourse.bass as bass
import concourse.mybir as mybir
from concourse.bass_utils import run_bass_kernel_spmd

BETA = 0.95
B, CH = 1024, 224
N_CORES = 8
PB = B // N_CORES  # 128 batches per core = partition dim
H = CH  # per-step vector length (contiguous, time-major)
W = CH  # time steps
SLAB = 16  # steps per DMA/compute slab
N_SLAB = W // SLAB
XRING = 4
MRING = 2
SRING = 4

_cache = {}


def _register_lif_op():
    """Register the custom DVE op (idempotent; per-process)."""
    import concourse.dve_ops as dve_ops
    from concourse.dve_spec import Spec, Src0, Src1, C0, C1, C2, lower, _has_src1
    from concourse.dve_uop import DveOpSpec

    name = "LIF_STEP_ANT"
    for o in dve_ops.OPS:
        if o.name == name:
            return o

    def _ref(in0, in1, s0, s1, imm2):
        m = in0.astype(np.float32)
        return (m * np.float32(s0) + in1.astype(np.float32)) - (
            (m > np.float32(s1)).astype(np.float32) * np.float32(imm2)
        )

    spec = Spec(body=(Src0 * C0 + Src1) - (Src0 > C1) * C2, reference=_ref)
    row = dve_ops._CUSTOM_DVE_ROW_BASE + len(dve_ops.OPS)
    op = dve_ops.DveOp(name, spec, subdim=False, uops_sha={})
    for ver in ("v3", "v4"):
        try:
            op.uops_sha[ver] = DveOpSpec(
                name=name, opcode=row, uops=lower(spec, ver=ver),
                rd1_en=_has_src1(spec),
            ).sha(ver)
        except Exception:
            pass  # only the version we run on (v3 for TRN2) is required
    dve_ops.OPS.append(op)
    dve_ops.CUSTOM_DVE_SPECS[name] = spec
    dve_ops._SUB_OPCODE_FOR_NAME[name] = row
    return op


def _build(T: float):
    lif = _register_lif_op()
    nc = bass.Bass(trn_type="TRN2")
    x_d = nc.dram_tensor("x", [PB, W * H], mybir.dt.float32, kind="ExternalInput")
    s_d = nc.dram_tensor("s", [PB, W * H], mybir.dt.uint8, kind="ExternalOutput")

    # [128,1] const holding -T, used as the Sign-activation bias (m - T).
    bias_t = nc.alloc_sbuf_tensor("const-neg-thresh", [128, 1], mybir.dt.float32)
    nc.gpsimd.memset(bias_t.ap(), float(-T))
    nc.const_aps.aps[(mybir.dt.float32, float(-T))] = bias_t.ap()
    nc.all_engine_barrier()

    L = SLAB * H  # free elems per slab

    with (
        nc.sbuf_tensor("xb0", [PB, L], mybir.dt.float32) as xb0,
        nc.sbuf_tensor("xb1", [PB, L], mybir.dt.float32) as xb1,
        nc.sbuf_tensor("xb2", [PB, L], mybir.dt.float32) as xb2,
        nc.sbuf_tensor("xb3", [PB, L], mybir.dt.float32) as xb3,
        nc.sbuf_tensor("mb0", [PB, L], mybir.dt.float32) as mb0,
        nc.sbuf_tensor("mb1", [PB, L], mybir.dt.float32) as mb1,
        nc.sbuf_tensor("sb0", [PB, L], mybir.dt.uint8) as sb0,
        nc.sbuf_tensor("sb1", [PB, L], mybir.dt.uint8) as sb1,
        nc.sbuf_tensor("sb2", [PB, L], mybir.dt.uint8) as sb2,
        nc.sbuf_tensor("sb3", [PB, L], mybir.dt.uint8) as sb3,
        nc.sbuf_tensor("zcol", [PB, H], mybir.dt.float32) as zcol,
        nc.semaphore() as in_sem,
        nc.semaphore() as v_sem,
        nc.semaphore() as a_sem,
        nc.semaphore() as out_sem,
        nc.Block() as block,
    ):
        xb = [xb0, xb1, xb2, xb3]
        mb = [mb0, mb1]
        sb = [sb0, sb1, sb2, sb3]

        @block.sync
        def _(sync):
            for s in range(N_SLAB):
                if s >= XRING:
                    # xb[s%XRING] free once DVE finished consuming slab s-XRING
                    sync.wait_ge(v_sem, s - XRING + 1)
                sync.dma_start(
                    xb[s % XRING][:, :], x_d[:, s * L:(s + 1) * L]
                ).then_inc(in_sem, 16)

        @block.vector
        def _(vector):
            nc.vector.memzero(zcol[:, :])
            for s in range(N_SLAB):
                vector.wait_ge(in_sem, 16 * (s + 1))
                if s >= MRING:
                    # mb[s%MRING] free once scalar read slab s-MRING (sign op)
                    vector.wait_ge(a_sem, s - MRING + 1)
                xt, mt = xb[s % XRING], mb[s % MRING]
                for tl in range(SLAB):
                    if s == 0 and tl == 0:
                        mprev = zcol[:, :]
                    elif tl == 0:
                        mprev = mb[(s - 1) % MRING][:, (SLAB - 1) * H:]
                    else:
                        mprev = mt[:, (tl - 1) * H: tl * H]
                    ins = nc.vector._custom_dve(
                        lif,
                        out=mt[:, tl * H:(tl + 1) * H],
                        in0=mprev,
                        in1=xt[:, tl * H:(tl + 1) * H],
                        s0=BETA, s1=T, imm2=T,
                    )
                    if tl == SLAB - 1:
                        ins.then_inc(v_sem, 1)

        @block.scalar
        def _(scalar):
            for s in range(N_SLAB):
                scalar.wait_ge(v_sem, s + 1)
                if s >= SRING:
                    # sb[s%SRING] free once its out-DMA (slab s-SRING) is done
                    scalar.wait_ge(out_sem, 16 * (s - SRING + 1))
                nc.scalar.sign(
                    sb[s % SRING][:, :], mb[s % MRING][:, :], bias=-T
                ).then_inc(a_sem, 1)
                # DMA trigger must not outrun the sign engine op
                scalar.wait_ge(a_sem, s + 1)
                scalar.dma_start(
                    s_d[:, s * L:(s + 1) * L], sb[s % SRING][:, :]
                ).then_inc(out_sem, 16)

    return nc


def kernel(inp: np.ndarray, t: np.ndarray, roll_amount) -> np.ndarray:
    T = float(
        np.clip(np.float32(np.asarray(t).reshape(-1)[0]), np.float32(1.0),
                np.float32(5.0))
    )
    roll = int(np.asarray(roll_amount)) % W

    key = (T,)
    if key not in _cache:
        _cache[key] = _build(T)
    nc = _cache[key]

    inp = np.asarray(inp, dtype=np.float32).reshape(B, CH, CH)
    in_maps = []
    for c in range(N_CORES):
        shard = inp[c * PB:(c + 1) * PB]  # (128, H, W)
        shard = np.roll(shard, roll, axis=2)
        # time-major: (128, W, H) contiguous
        x_tm = np.ascontiguousarray(shard.transpose(0, 2, 1)).reshape(PB, W * H)
        in_maps.append({"x": x_tm})

    res = run_bass_kernel_spmd(nc, in_maps, core_ids=list(range(N_CORES)))

    out = np.empty((B, 1, CH, CH), dtype=np.float32)
    for c in range(N_CORES):
        sp = res.results[c]["s"].reshape(PB, W, H)  # (b, w, h) uint8
        # spike <=> Sign(m - T) == +1 (uint8 1 under either saturating or
        # wrapping float->u8 conversion of {-1, 0, +1})
        out[c * PB:(c + 1) * PB, 0] = (sp == 1).transpose(0, 2, 1)
    return out


# revision 4
# speedup vs baseline: 1.8078x; 1.8078x over previous
"""Trainium2 Bass kernel for nn_CT_37821482009425 (snntorch Leaky LIF scan).

Reference semantics (bitwise-matched):
    T = clip(t, 1, 5); x = roll(inp, roll_amount, axis=2)
    per step: reset = (mem > T); mem = 0.95*mem + x_t - reset*T; spk = (mem > T)
Output: spikes (1024, 1, 224, 224) float32 in {0, 1}.

Distribution: pure data parallelism - batch 1024 -> 8 cores x 128 (the SBUF
partition dim). Host prep per core: apply the roll and transpose to
time-major so each timestep's H=224 vector is contiguous per partition.

Per-core compute:
  * One custom DVE op per time step (LIF_STEP_ANT):
        out = (Src0*C0 + Src1) - (Src0 > C1)*C2
             = beta*m_prev + x_t - T*(m_prev > T)
    The reset feedback is re-derived from the membrane inside the op, so the
    whole serial step is a single 2-source DVE instruction (~294 ns at
    FD=224) instead of the 3-instruction stt/tt/ts chain. Rounding order
    matches the reference exactly (mul, add, exact mask*T, sub).
  * Spikes are extracted off the critical path on the scalar engine:
    Sign(m - T) written as uint8 per 16-step slab (spike <=> byte == 1),
    quartering output DMA traffic (input and output transfers serialize on
    the DMA engines, so output bytes are on the critical path).
  * sync engine streams x in 16-step slabs (4-deep ring), scalar engine
    writes spike slabs out (4-deep ring); membrane slabs ping-pong (x2).
"""

import numpy as np
import concourse.bass as bass
import concourse.mybir as mybir
from concourse.bass_utils import run_bass_kernel_spmd

BETA = 0.95
B, CH = 1024, 224
N_CORES = 8
PB = B // N_CORES  # 128 batches per core = partition dim
H = CH  # per-step vector length (contiguous, time-major)
W = CH  # time steps
SLAB = 16  # steps per DMA/compute slab
N_SLAB = W // SLAB
XRING = 4
MRING = 2
SRING = 4

_cache = {}


def _register_lif_op():
    """Register the custom DVE op (idempotent; per-process)."""
    import concourse.dve_ops as dve_ops
    from concourse.dve_spec import Spec, Src0, Src1, C0, C1, C2, lower, _has_src1
    from concourse.dve_uop import DveOpSpec

    name = "LIF_STEP_ANT"
    for o in dve_ops.OPS:
        if o.name == name:
            return o

    def _ref(in0, in1, s0, s1, imm2):
        m = in0.astype(np.float32)
        return (m * np.float32(s0) + in1.astype(np.float32)) - (
            (m > np.float32(s1)).astype(np.float32) * np.float32(imm2)
        )

    spec = Spec(body=(Src0 * C0 + Src1) - (Src0 > C1) * C2, reference=_ref)
    row = dve_ops._CUSTOM_DVE_ROW_BASE + len(dve_ops.OPS)
    op = dve_ops.DveOp(name, spec, subdim=False, uops_sha={})
    for ver in ("v3", "v4"):
        try:
            op.uops_sha[ver] = DveOpSpec(
                name=name, opcode=row, uops=lower(spec, ver=ver),
                rd1_en=_has_src1(spec),
            ).sha(ver)
        except Exception:
            pass  # only the version we run on (v3 for TRN2) is required
    dve_ops.OPS.append(op)
    dve_ops.CUSTOM_DVE_SPECS[name] = spec
    dve_ops._SUB_OPCODE_FOR_NAME[name] = row
    return op


def _build(T: float):
    lif = _register_lif_op()
    nc = bass.Bass(trn_type="TRN2")
    x_d = nc.dram_tensor("x", [PB, W * H], mybir.dt.float32, kind="ExternalInput")
    s_d = nc.dram_tensor("s", [PB, W * H], mybir.dt.uint8, kind="ExternalOutput")

    # [128,1] const holding -T, used as the Sign-activation bias (m - T).
    bias_t = nc.alloc_sbuf_tensor("const-neg-thresh", [128, 1], mybir.dt.float32)
    nc.gpsimd.memset(bias_t.ap(), float(-T))
    nc.const_aps.aps[(mybir.dt.float32, float(-T))] = bias_t.ap()
    nc.all_engine_barrier()

    L = SLAB * H  # free elems per slab

    with (
        nc.sbuf_tensor("xb0", [PB, L], mybir.dt.float32) as xb0,
        nc.sbuf_tensor("xb1", [PB, L], mybir.dt.float32) as xb1,
        nc.sbuf_tensor("xb2", [PB, L], mybir.dt.float32) as xb2,
        nc.sbuf_tensor("xb3", [PB, L], mybir.dt.float32) as xb3,
        nc.sbuf_tensor("mb0", [PB, L], mybir.dt.float32) as mb0,
        nc.sbuf_tensor("mb1", [PB, L], mybir.dt.float32) as mb1,
        nc.sbuf_tensor("sb0", [PB, L], mybir.dt.uint8) as sb0,
        nc.sbuf_tensor("sb1", [PB, L], mybir.dt.uint8) as sb1,
        nc.sbuf_tensor("sb2", [PB, L], mybir.dt.uint8) as sb2,
        nc.sbuf_tensor("sb3", [PB, L], mybir.dt.uint8) as sb3,
        nc.sbuf_tensor("zcol", [PB, H], mybir.dt.float32) as zcol,
        nc.semaphore() as in_sem,
        nc.semaphore() as v_sem,
        nc.semaphore() as a_sem,
        nc.semaphore() as out_sem,
        nc.Block() as block,
    ):
        xb = [xb0, xb1, xb2, xb3]
        mb = [mb0, mb1]
        sb = [sb0, sb1, sb2, sb3]

        @block.sync
        def _(sync):
            for s in range(N_SLAB):
                if s >= XRING:
                    # xb[s%XRING] free once DVE finished consuming slab s-XRING
                    sync.wait_ge(v_sem, s - XRING + 1)
                sync.dma_start(
                    xb[s % XRING][:, :], x_d[:, s * L:(s + 1) * L]
                ).then_inc(in_sem, 16)

        @block.vector
        def _(vector):
            nc.vector.memzero(zcol[:, :])
            for s in range(N_SLAB):
                vector.wait_ge(in_sem, 16 * (s + 1))
                if s >= MRING:
                    # mb[s%MRING] free once scalar read slab s-MRING (sign op)
                    vector.wait_ge(a_sem, s - MRING + 1)
                xt, mt = xb[s % XRING], mb[s % MRING]
                for tl in range(SLAB):
                    if s == 0 and tl == 0:
                        mprev = zcol[:, :]
                    elif tl == 0:
                        mprev = mb[(s - 1) % MRING][:, (SLAB - 1) * H:]
                    else:
                        mprev = mt[:, (tl - 1) * H: tl * H]
                    ins = nc.vector._custom_dve(
                        lif,
                        out=mt[:, tl * H:(tl + 1) * H],
                        in0=mprev,
                        in1=xt[:, tl * H:(tl + 1) * H],
                        s0=BETA, s1=T, imm2=T,
                    )
                    if tl == SLAB - 1:
                        ins.then_inc(v_sem, 1)

        @block.scalar
        def _(scalar):
            for s in range(N_SLAB):
                scalar.wait_ge(v_sem, s + 1)
                if s >= SRING:
                    # sb[s%SRING] free once its out-DMA (slab s-SRING) is done
                    scalar.wait_ge(out_sem, 16 * (s - SRING + 1))
                nc.scalar.sign(
                    sb[s % SRING][:, :], mb[s % MRING][:, :], bias=-T
                ).then_inc(a_sem, 1)
                # DMA trigger must not outrun the sign engine op
                scalar.wait_ge(a_sem, s + 1)
                scalar.dma_start(
                    s_d[:, s * L:(s + 1) * L], sb[s % SRING][:, :]
                ).then_inc(out_sem, 16)

    # Populate .instr bytes for the InstCustomDveAnt (InstISA subclass);
    # without this the NEFF compiler fails with "ISA wrong length".
    mybir.codegen_inst_isa_subclasses(nc)
    return nc


def kernel(inp: np.ndarray, t: np.ndarray, roll_amount) -> np.ndarray:
    T = float(
        np.clip(np.float32(np.asarray(t).reshape(-1)[0]), np.float32(1.0),
                np.float32(5.0))
    )
    roll = int(np.asarray(roll_amount)) % W

    key = (T,)
    if key not in _cache:
        _cache[key] = _build(T)
    nc = _cache[key]

    inp = np.asarray(inp, dtype=np.float32).reshape(B, CH, CH)
    in_maps = []
    for c in range(N_CORES):
        shard = inp[c * PB:(c + 1) * PB]  # (128, H, W)
        shard = np.roll(shard, roll, axis=2)
        # time-major: (128, W, H) contiguous
        x_tm = np.ascontiguousarray(shard.transpose(0, 2, 1)).reshape(PB, W * H)
        in_maps.append({"x": x_tm})

    res = run_bass_kernel_spmd(nc, in_maps, core_ids=list(range(N_CORES)))

    out = np.empty((B, 1, CH, CH), dtype=np.float32)
    for c in range(N_CORES):
        sp = res.results[c]["s"].reshape(PB, W, H)  # (b, w, h) uint8
        # spike <=> Sign(m - T) == +1 (uint8 1 under either saturating or
        # wrapping float->u8 conversion of {-1, 0, +1})
        out[c * PB:(c + 1) * PB, 0] = (sp == 1).transpose(0, 2, 1)
    return out
